# revision 3
# baseline (speedup 1.0000x reference)
"""Trainium2 Bass kernel for nn_Net_37048387896011.

Three stacked 3x3 SAME convs (3->16->16->16 channels) + final ReLU on a
[4, 3, 1536, 1536] fp32 image, distributed over 8 NeuronCores as
(batch x H-halves).  No collectives: each core's input slab carries a
3-row halo read from the full image on the host.

Math mapping (per conv layer): multi-pixel Toeplitz matmul.  The
stationary operand is [K=(t,ci)=128, M=(p,cout)=96] where t indexes an
8-wide input window along W and p indexes 6 output pixels; kernel rows
(kh) are accumulated into PSUM over 3 matmuls (conv1 instead folds kh
into K=72 using host-replicated input rows, one matmul per block).
Activations live in SBUF in a Toeplitz layout: the partition for (t,ci)
holds x[ci, :, 6j+t-1] over j-blocks of 6 pixels, so one matmul column
computes 6 output pixels x 16 channels with no on-device im2col.

SBUF layout per activation buffer [128, rows, JW+2]:
  partitions [0,96)    = t in 1..6 at (t-1)*16+ci   (written by PSUM evac
                         at partition offset 0 - engine APs must start at
                         partition 0/32/64/96)
  partitions [96,112)  = t=0   (dup of t=6 shifted one j-block)
  partitions [112,128) = t=7   (dup of t=1 shifted one j-block)
  free cols 0 and JW+1 are permanent zeros; j-data lives in [1, JW+1).
The dup groups are filled by two SBUF->SBUF DMAs per buffer per tile
(DMA is not subject to the partition-quadrant constraint), and the zero
columns make the W-edge SAME-padding zeros automatic.

The device writes conv3's output in its natural PSUM order
[(p,cout), h, j]; the host de-interleaves to [cout, h, w] and patches
the two image-edge rows (h=0, h=1535) whose device values saw a garbage
SAME-pad halo row.
"""

import sys

if "/opt/trn_rl_repo" not in sys.path:
    sys.path.insert(0, "/opt/trn_rl_repo")

import ml_dtypes
import numpy as np

BF16 = ml_dtypes.bfloat16

FULL_CFG = dict(
    B=4, CIN=3, CC=16, H=1536, W=1536,
    SLAB=768,     # output rows per core
    R=64,         # output rows per tile
    OC=16,        # output rows per out-DMA chunk
    N_CORES=8,
)

P6 = 6   # output pixels per matmul column
T8 = 8   # input window width (P6 + 2)

# SBUF partition row for Toeplitz slot t (times 16, plus ci):
#   t=1..6 -> 0..5 ;  t=0 -> 6 ;  t=7 -> 7
SROW = {1: 0, 2: 1, 3: 2, 4: 3, 5: 4, 6: 5, 0: 6, 7: 7}


def _build_program(cfg):
    """Build the (SPMD-identical) Bass program for one core."""
    import concourse.bacc as bacc
    import concourse.mybir as mybir
    from concourse.tile import TileContext

    W, SLAB, R, OC = cfg["W"], cfg["SLAB"], cfg["R"], cfg["OC"]
    CIN, CC = cfg["CIN"], cfg["CC"]
    JW = W // P6
    NT = SLAB // R
    K1 = 3 * T8 * CIN          # conv1 contraction: (kh, t, ci)
    M = P6 * CC                # matmul output: (p, co)
    XROWS = SLAB + 4           # conv1-output rows per core

    f32 = mybir.dt.float32
    bf16 = mybir.dt.bfloat16

    nc = bacc.Bacc(None, target_bir_lowering=False, debug=False)

    xT_d = nc.declare_dram_parameter("xT", [K1, XROWS, JW], bf16, isOutput=False)
    s1_d = nc.declare_dram_parameter("s1", [K1, M], bf16, isOutput=False)
    s2_d = nc.declare_dram_parameter("s2", [T8 * CC, 3, M], bf16, isOutput=False)
    s3_d = nc.declare_dram_parameter("s3", [T8 * CC, 3, M], bf16, isOutput=False)
    out_d = nc.declare_dram_parameter("out", [M, SLAB, JW], f32, isOutput=True)

    with TileContext(nc) as tc:
        with (
            tc.tile_pool(name="weights", bufs=1) as wpool,
            tc.tile_pool(name="xt", bufs=2) as xtpool,
            tc.tile_pool(name="act", bufs=1) as actpool,
            tc.tile_pool(name="outsb", bufs=3) as outpool,
            tc.tile_pool(name="ps", bufs=8, space="PSUM") as pspool,
        ):
            s1_sb = wpool.tile([K1, M], bf16, tag="s1")
            nc.sync.dma_start(out=s1_sb[:], in_=s1_d[:])
            s2_sb = wpool.tile([T8 * CC, 3, M], bf16, tag="s2")
            nc.sync.dma_start(out=s2_sb[:], in_=s2_d[:])
            s3_sb = wpool.tile([T8 * CC, 3, M], bf16, tag="s3")
            nc.sync.dma_start(out=s3_sb[:], in_=s3_d[:])

            a1 = actpool.tile([T8 * CC, R + 4, JW + 2], bf16, tag="a1")
            a2 = actpool.tile([T8 * CC, R + 2, JW + 2], bf16, tag="a2")
            # permanent zero columns (cols 0 and JW+1 of the main groups;
            # only (t=6, col 0) and (t=1, col JW+1) are ever read, via the
            # tail DMAs below, but zeroing the whole main group is the
            # partition-legal op)
            for a in (a1, a2):
                nc.gpsimd.memset(a[0 : 6 * CC, :, 0:1], 0.0)
                nc.gpsimd.memset(a[0 : 6 * CC, :, JW + 1 : JW + 2], 0.0)

            def fill_dups(a, rows):
                # t=0 slots [96,112): a[t0, r, 1+j] = a[t6, r, j]   (j-1 shift)
                nc.sync.dma_start(
                    out=a[6 * CC : 7 * CC, 0:rows, 1 : JW + 1],
                    in_=a[5 * CC : 6 * CC, 0:rows, 0:JW],
                )
                # t=7 slots [112,128): a[t7, r, 1+j] = a[t1, r, 2+j] (j+1 shift)
                nc.sync.dma_start(
                    out=a[7 * CC : 8 * CC, 0:rows, 1 : JW + 1],
                    in_=a[0:CC, 0:rows, 2 : JW + 2],
                )

            for g in range(NT):
                gb = g * R

                xt = xtpool.tile([K1, R + 4, JW], bf16, tag="xt")
                nc.sync.dma_start(out=xt[:], in_=xT_d[:, gb : gb + R + 4, :])

                # ---- conv1: one matmul per 2-row block (kh folded in K) --
                for i in range((R + 4) // 2):
                    r0 = 2 * i
                    ps = pspool.tile([M, 2, JW], f32, tag="ps")
                    nc.tensor.matmul(
                        ps[:], lhsT=s1_sb[:], rhs=xt[:, r0 : r0 + 2, :],
                        start=True, stop=True,
                    )
                    # psum (p,co) -> partition p*16+co (t=p+1 slot)
                    nc.vector.tensor_copy(
                        a1[0 : 6 * CC, r0 : r0 + 2, 1 : JW + 1], ps[:]
                    )
                fill_dups(a1, R + 4)

                # ---- conv2: 3 kh-accumulated matmuls per block ----------
                nblk2 = (R + 2) // 2
                for base in range(0, nblk2, 6):
                    blks = list(range(base, min(base + 6, nblk2)))
                    pss = [
                        pspool.tile([M, 2, JW], f32, tag="ps",
                                    name=f"psb_{g}_{base}_{_i}")
                        for _i in range(len(blks))
                    ]
                    for kh in range(3):
                        for ps, i in zip(pss, blks):
                            nc.tensor.matmul(
                                ps[:], lhsT=s2_sb[:, kh, :],
                                rhs=a1[:, 2 * i + kh : 2 * i + kh + 2, 1 : JW + 1],
                                start=(kh == 0), stop=(kh == 2),
                            )
                    for ps, i in zip(pss, blks):
                        nc.vector.tensor_copy(
                            a2[0 : 6 * CC, 2 * i : 2 * i + 2, 1 : JW + 1], ps[:]
                        )
                fill_dups(a2, R + 2)

                # ---- conv3 + relu -> out staging -> DRAM ----------------
                for s in range(R // OC):
                    osb = outpool.tile([M, OC, JW], f32, tag="outsb")
                    nblk = OC // 2
                    for base in range(0, nblk, 4):
                        blks = list(range(base, min(base + 4, nblk)))
                        pss = [
                            pspool.tile([M, 2, JW], f32, tag="ps",
                                        name=f"psc_{g}_{s}_{base}_{_i}")
                            for _i in range(len(blks))
                        ]
                        for kh in range(3):
                            for ps, i in zip(pss, blks):
                                ii = s * nblk + i
                                nc.tensor.matmul(
                                    ps[:], lhsT=s3_sb[:, kh, :],
                                    rhs=a2[:, 2 * ii + kh : 2 * ii + kh + 2,
                                           1 : JW + 1],
                                    start=(kh == 0), stop=(kh == 2),
                                )
                        for ps, i in zip(pss, blks):
                            nc.scalar.activation(
                                osb[:, 2 * i : 2 * i + 2, :], ps[:],
                                mybir.ActivationFunctionType.Relu,
                            )
                    nc.sync.dma_start(
                        out=out_d[:, gb + s * OC : gb + (s + 1) * OC, :],
                        in_=osb[:],
                    )

    nc.compile()
    return nc


# ------------------------------------------------------------ host prep

def _make_stationaries(w1, w2, w3, cfg):
    CIN, CC = cfg["CIN"], cfg["CC"]
    M = P6 * CC
    K1 = 3 * T8 * CIN

    # conv1: natural (kh, t, ci) K-order, matching xT
    s1 = np.zeros((K1, M), np.float32)
    for kh in range(3):
        for t in range(T8):
            for p in range(P6):
                kw = t - p
                if 0 <= kw <= 2:
                    s1[
                        kh * T8 * CIN + t * CIN : kh * T8 * CIN + (t + 1) * CIN,
                        p * CC : (p + 1) * CC,
                    ] = w1[:, :, kh, kw].T

    # conv2/3: SROW-permuted K-order matching the SBUF Toeplitz layout
    def mk_s(w):
        s = np.zeros((T8 * CC, 3, M), np.float32)
        for kh in range(3):
            for t in range(T8):
                r = SROW[t] * CC
                for p in range(P6):
                    kw = t - p
                    if 0 <= kw <= 2:
                        s[r : r + CC, kh, p * CC : (p + 1) * CC] = (
                            w[:, :, kh, kw].T
                        )
        return s

    return s1.astype(BF16), mk_s(w2).astype(BF16), mk_s(w3).astype(BF16)


def _make_xT(xb, h0, cfg):
    """Per-core conv1 rhs: xT[(kh,t,ci), r1, j] = x[ci, h0-3+r1+kh, 6j+t-1],
    zero outside the image; rows r1 whose conv1-output image row is out of
    range are fully zeroed (so conv1 emits zero rows, matching the SAME-pad
    semantics of the following conv)."""
    CIN, H, W, SLAB = cfg["CIN"], cfg["H"], cfg["W"], cfg["SLAB"]
    JW = W // P6
    XROWS = SLAB + 4
    xp = np.zeros((CIN, XROWS + 2, W + T8), np.float32)
    rlo = max(0, h0 - 3)
    rhi = min(H, h0 + SLAB + 3)
    xp[:, rlo - (h0 - 3) : rhi - (h0 - 3), 1 : 1 + W] = xb[:, rlo:rhi, :]

    xT = np.empty((3, T8, CIN, XROWS, JW), np.float32)
    for kh in range(3):
        for t in range(T8):
            # w = 6j + t - 1  ->  padded col index 6j + t
            xT[kh, t] = xp[:, kh : kh + XROWS, t : t + 6 * JW : 6]
    xT = xT.reshape(3 * T8 * CIN, XROWS, JW)
    if h0 == 0:
        xT[:, 0:2, :] = 0.0
    if h0 + SLAB == H:
        xT[:, XROWS - 2 : XROWS, :] = 0.0
    return np.ascontiguousarray(xT).astype(BF16)


def _np_conv3x3(x, w):
    """Valid-in-H, SAME-in-W 3x3 conv.  x [ci, Rin, W] -> [co, Rin-2, W]."""
    ci, rin, W = x.shape
    xp = np.pad(x, ((0, 0), (0, 0), (1, 1)))
    out = np.zeros((w.shape[0], rin - 2, W), np.float32)
    for kh in range(3):
        for kw in range(3):
            out += np.einsum(
                "oi,irw->orw", w[:, :, kh, kw], xp[:, kh : kh + rin - 2, kw : kw + W]
            )
    return out


def _patch_edge_row(xb, w1, w2, w3, row, H):
    """Host-recompute output row `row` (0 or H-1) of one batch image."""
    strip = np.zeros((xb.shape[0], 7, xb.shape[2]), np.float32)
    if row == 0:
        strip[:, 3:7] = xb[:, 0:4]
        a1 = _np_conv3x3(strip, w1)        # rows -2..2
        a1[:, 0:2] = 0.0                   # SAME-pad zeros of conv2's input
        a2 = _np_conv3x3(a1, w2)           # rows -1..1
        a2[:, 0:1] = 0.0
        o = _np_conv3x3(a2, w3)            # row 0
    else:
        strip[:, 0:4] = xb[:, H - 4 : H]
        a1 = _np_conv3x3(strip, w1)        # rows H-3..H+1
        a1[:, 3:5] = 0.0
        a2 = _np_conv3x3(a1, w2)           # rows H-2..H
        a2[:, 2:3] = 0.0
        o = _np_conv3x3(a2, w3)            # row H-1
    return np.maximum(o[:, 0], 0.0)


# ------------------------------------------------------------- runners

def _install_ntff_hook():
    """Provide antenv.axon_hooks (absent in this image) so bass_utils can
    profile through the axon NTFF path."""
    import types

    if "antenv.axon_hooks" in sys.modules:
        return
    try:
        from trn_agent_boot.trn_boot import _ntff_profile_via_ctypes

        hook = _ntff_profile_via_ctypes("/opt/axon/libaxon_pjrt.so")
    except Exception:
        hook = None
    mod = types.ModuleType("antenv.axon_hooks")
    mod.get_axon_ntff_profile_hook = lambda: hook
    mod.set_axon_ntff_profile_hook = lambda h: None
    sys.modules["antenv.axon_hooks"] = mod


_PROGRAM_CACHE = {}


def _get_program():
    if "full" not in _PROGRAM_CACHE:
        _PROGRAM_CACHE["full"] = _build_program(FULL_CFG)
    return _PROGRAM_CACHE["full"]


def _make_in_maps(x, w1, w2, w3, cfg):
    SLAB, H = cfg["SLAB"], cfg["H"]
    halves = H // SLAB
    s1, s2, s3 = _make_stationaries(w1, w2, w3, cfg)
    in_maps = []
    for core in range(cfg["N_CORES"]):
        b, half = core // halves, core % halves
        in_maps.append(
            {"xT": _make_xT(x[b], half * SLAB, cfg), "s1": s1, "s2": s2, "s3": s3}
        )
    return in_maps


def _assemble(res, x, w1, w2, w3, cfg):
    B, H, W, SLAB, CC = cfg["B"], cfg["H"], cfg["W"], cfg["SLAB"], cfg["CC"]
    JW = W // P6
    halves = H // SLAB
    y = np.empty((B, CC, H, W), np.float32)
    for core in range(cfg["N_CORES"]):
        b, half = core // halves, core % halves
        h0 = half * SLAB
        r = np.asarray(res[core]["out"])  # [(p,co)=96, SLAB, JW]
        r = (
            r.reshape(P6, CC, SLAB, JW)
            .transpose(1, 2, 3, 0)
            .reshape(CC, SLAB, JW * P6)
        )
        y[b, :, h0 : h0 + SLAB, :] = r
    for b in range(B):
        y[b, :, 0, :] = _patch_edge_row(x[b], w1, w2, w3, 0, H)
        y[b, :, H - 1, :] = _patch_edge_row(x[b], w1, w2, w3, H - 1, H)
    return y


def _kernel_impl(x, w1, w2, w3, cfg, runner):
    in_maps = _make_in_maps(x, w1, w2, w3, cfg)
    res = runner(in_maps)
    return _assemble(res, x, w1, w2, w3, cfg)


def kernel(x, w1, w2, w3, H, W, Th, Tw):
    x = np.asarray(x, dtype=np.float32)
    w1 = np.asarray(w1, dtype=np.float32)
    w2 = np.asarray(w2, dtype=np.float32)
    w3 = np.asarray(w3, dtype=np.float32)
    cfg = dict(FULL_CFG)
    assert x.shape == (cfg["B"], cfg["CIN"], cfg["H"], cfg["W"]), x.shape

    nc = _get_program()

    def runner(in_maps):
        from concourse.bass_utils import run_bass_kernel_spmd

        return run_bass_kernel_spmd(
            nc, in_maps, list(range(cfg["N_CORES"]))
        ).results

    return _kernel_impl(x, w1, w2, w3, cfg, runner)


# revision 4
# speedup vs baseline: 1.2560x; 1.2560x over previous
"""Trainium2 Bass kernel for nn_Net_37048387896011.

Three stacked 3x3 SAME convs (3->16->16->16 channels) + final ReLU on a
[4, 3, 1536, 1536] fp32 image, distributed over 8 NeuronCores as
(batch x H-halves).  No collectives: each core's input slab carries a
3-row halo read from the full image on the host.

Math mapping (per conv layer): multi-pixel Toeplitz matmul.  The
stationary operand is [K=(t,ci)=128, M=(p,cout)=96] where t indexes an
8-wide input window along W and p indexes 6 output pixels; kernel rows
(kh) are accumulated into PSUM over 3 matmuls (conv1 instead folds kh
into K=72 using host-replicated input rows, one matmul per block).
Activations live in SBUF in a Toeplitz layout: the partition for (t,ci)
holds x[ci, :, 6j+t-1] over j-blocks of 6 pixels, so one matmul column
computes 6 output pixels x 16 channels with no on-device im2col.

SBUF layout per activation buffer [128, rows, JW+2]:
  partitions [0,96)    = t in 1..6 at (t-1)*16+ci   (written by PSUM evac
                         at partition offset 0 - engine APs must start at
                         partition 0/32/64/96)
  partitions [96,112)  = t=0   (dup of t=6 shifted one j-block)
  partitions [112,128) = t=7   (dup of t=1 shifted one j-block)
  free cols 0 and JW+1 are permanent zeros; j-data lives in [1, JW+1).
The dup groups are filled by two SBUF->SBUF DMAs per buffer per tile
(DMA is not subject to the partition-quadrant constraint), and the zero
columns make the W-edge SAME-padding zeros automatic.

The device writes conv3's output in its natural PSUM order
[(p,cout), h, j]; the host de-interleaves to [cout, h, w] and patches
the two image-edge rows (h=0, h=1535) whose device values saw a garbage
SAME-pad halo row.
"""

import sys

if "/opt/trn_rl_repo" not in sys.path:
    sys.path.insert(0, "/opt/trn_rl_repo")

import ml_dtypes
import numpy as np

BF16 = ml_dtypes.bfloat16

FULL_CFG = dict(
    B=4, CIN=3, CC=16, H=1536, W=1536,
    SLAB=768,     # output rows per core
    R=48,         # output rows per tile
    OC=8,         # output rows per out-DMA chunk
    N_CORES=8,
)

P6 = 6   # output pixels per matmul column
T8 = 8   # input window width (P6 + 2)

# SBUF partition row for Toeplitz slot t (times 16, plus ci):
#   t=1..6 -> 0..5 ;  t=0 -> 6 ;  t=7 -> 7
SROW = {1: 0, 2: 1, 3: 2, 4: 3, 5: 4, 6: 5, 0: 6, 7: 7}


def _build_program(cfg):
    """Build the (SPMD-identical) Bass program for one core."""
    import concourse.bacc as bacc
    import concourse.mybir as mybir
    from concourse.tile import TileContext

    W, SLAB, R, OC = cfg["W"], cfg["SLAB"], cfg["R"], cfg["OC"]
    CIN, CC = cfg["CIN"], cfg["CC"]
    JW = W // P6
    NT = SLAB // R
    K1 = 3 * T8 * CIN          # conv1 contraction: (kh, t, ci)
    M = 128                    # matmul output: (p, co) = 96, padded to 128
                               # so bf16 Fast Weight Load engages
    MOUT = P6 * CC             # the 96 meaningful output partitions
    XROWS = SLAB + 4           # conv1-output rows per core

    f32 = mybir.dt.float32
    bf16 = mybir.dt.bfloat16

    nc = bacc.Bacc(None, target_bir_lowering=False, debug=False)

    xT_d = nc.declare_dram_parameter("xT", [K1, XROWS, JW], bf16, isOutput=False)
    s1_d = nc.declare_dram_parameter("s1", [K1, M], bf16, isOutput=False)
    s2_d = nc.declare_dram_parameter("s2", [T8 * CC, 3, M], bf16, isOutput=False)
    s3_d = nc.declare_dram_parameter("s3", [T8 * CC, 3, M], bf16, isOutput=False)
    out_d = nc.declare_dram_parameter("out", [MOUT, SLAB, JW], f32, isOutput=True)

    with TileContext(nc) as tc:
        with (
            tc.tile_pool(name="weights", bufs=1) as wpool,
            tc.tile_pool(name="xt", bufs=2) as xtpool,
            tc.tile_pool(name="act", bufs=1) as actpool,
            tc.tile_pool(name="outsb", bufs=3) as outpool,
            tc.tile_pool(name="ps", bufs=8, space="PSUM") as pspool,
        ):
            s1_sb = wpool.tile([K1, M], bf16, tag="s1")
            nc.sync.dma_start(out=s1_sb[:], in_=s1_d[:])
            s2_sb = wpool.tile([T8 * CC, 3, M], bf16, tag="s2")
            nc.sync.dma_start(out=s2_sb[:], in_=s2_d[:])
            s3_sb = wpool.tile([T8 * CC, 3, M], bf16, tag="s3")
            nc.sync.dma_start(out=s3_sb[:], in_=s3_d[:])

            # double-buffered a1 (conv1 of tile g+1 overlaps conv2/3 of g)
            a1A = actpool.tile([T8 * CC, R + 4, JW + 2], bf16, tag="a1A")
            a1B = actpool.tile([T8 * CC, R + 4, JW + 2], bf16, tag="a1B")
            a2 = actpool.tile([T8 * CC, R + 2, JW + 2], bf16, tag="a2")
            for a in (a1A, a1B, a2):
                nc.gpsimd.memset(a[0 : 6 * CC, :, 0:1], 0.0)
                nc.gpsimd.memset(a[0 : 6 * CC, :, JW + 1 : JW + 2], 0.0)

            def fill_dups(a, rows):
                # Issued on the ACT HWDGE ring, separate from the bulk
                # (sync-ring) DMAs, so these latency-critical fills do not
                # queue behind megabyte transfers.
                # t=0 slots [96,112): a[t0, r, 1+j] = a[t6, r, j]   (j-1)
                nc.scalar.dma_start(
                    out=a[6 * CC : 7 * CC, 0:rows, 1 : JW + 1],
                    in_=a[5 * CC : 6 * CC, 0:rows, 0:JW],
                )
                # t=7 slots [112,128): a[t7, r, 1+j] = a[t1, r, 2+j] (j+1)
                nc.scalar.dma_start(
                    out=a[7 * CC : 8 * CC, 0:rows, 1 : JW + 1],
                    in_=a[0:CC, 0:rows, 2 : JW + 2],
                )

            def conv1_tile(g, a1):
                gb = g * R
                xt = xtpool.tile([K1, R + 4, JW], bf16, tag="xt",
                                 name=f"xt_{g}")
                nc.sync.dma_start(out=xt[:], in_=xT_d[:, gb : gb + R + 4, :])
                for i in range((R + 4) // 2):
                    r0 = 2 * i
                    ps = pspool.tile([M, 2, JW], f32, tag="ps",
                                     name=f"psa_{g}_{i}")
                    nc.tensor.matmul(
                        ps[:], lhsT=s1_sb[:], rhs=xt[:, r0 : r0 + 2, :],
                        start=True, stop=True,
                    )
                    # psum (p,co) -> partition p*16+co (t=p+1 slot)
                    nc.vector.tensor_copy(
                        a1[0 : 6 * CC, r0 : r0 + 2, 1 : JW + 1], ps[0:MOUT]
                    )
                fill_dups(a1, R + 4)

            def conv23_tile(g, a1):
                gb = g * R
                nblk2 = (R + 2) // 2
                for base in range(0, nblk2, 6):
                    blks = list(range(base, min(base + 6, nblk2)))
                    pss = [
                        pspool.tile([M, 2, JW], f32, tag="ps",
                                    name=f"psb_{g}_{base}_{_i}")
                        for _i in range(len(blks))
                    ]
                    for kh in range(3):
                        for ps, i in zip(pss, blks):
                            nc.tensor.matmul(
                                ps[:], lhsT=s2_sb[:, kh, :],
                                rhs=a1[:, 2 * i + kh : 2 * i + kh + 2, 1 : JW + 1],
                                start=(kh == 0), stop=(kh == 2),
                            )
                    for ps, i in zip(pss, blks):
                        nc.vector.tensor_copy(
                            a2[0 : 6 * CC, 2 * i : 2 * i + 2, 1 : JW + 1],
                            ps[0:MOUT],
                        )
                fill_dups(a2, R + 2)

                for s in range(R // OC):
                    osb = outpool.tile([MOUT, OC, JW], f32, tag="outsb",
                                       name=f"osb_{g}_{s}")
                    nblk = OC // 2
                    for base in range(0, nblk, 4):
                        blks = list(range(base, min(base + 4, nblk)))
                        pss = [
                            pspool.tile([M, 2, JW], f32, tag="ps",
                                        name=f"psc_{g}_{s}_{base}_{_i}")
                            for _i in range(len(blks))
                        ]
                        for kh in range(3):
                            for ps, i in zip(pss, blks):
                                ii = s * nblk + i
                                nc.tensor.matmul(
                                    ps[:], lhsT=s3_sb[:, kh, :],
                                    rhs=a2[:, 2 * ii + kh : 2 * ii + kh + 2,
                                           1 : JW + 1],
                                    start=(kh == 0), stop=(kh == 2),
                                )
                        for ps, i in zip(pss, blks):
                            nc.scalar.activation(
                                osb[:, 2 * i : 2 * i + 2, :], ps[0:MOUT],
                                mybir.ActivationFunctionType.Relu,
                            )
                    nc.sync.dma_start(
                        out=out_d[:, gb + s * OC : gb + (s + 1) * OC, :],
                        in_=osb[:],
                    )

            a1bufs = [a1A, a1B]
            conv1_tile(0, a1bufs[0])
            for g in range(NT):
                if g + 1 < NT:
                    conv1_tile(g + 1, a1bufs[(g + 1) % 2])
                conv23_tile(g, a1bufs[g % 2])

    nc.compile()
    return nc


# ------------------------------------------------------------ host prep

def _make_stationaries(w1, w2, w3, cfg):
    CIN, CC = cfg["CIN"], cfg["CC"]
    M = 128
    K1 = 3 * T8 * CIN

    # conv1: natural (kh, t, ci) K-order, matching xT
    s1 = np.zeros((K1, M), np.float32)
    for kh in range(3):
        for t in range(T8):
            for p in range(P6):
                kw = t - p
                if 0 <= kw <= 2:
                    s1[
                        kh * T8 * CIN + t * CIN : kh * T8 * CIN + (t + 1) * CIN,
                        p * CC : (p + 1) * CC,
                    ] = w1[:, :, kh, kw].T

    # conv2/3: SROW-permuted K-order matching the SBUF Toeplitz layout
    def mk_s(w):
        s = np.zeros((T8 * CC, 3, M), np.float32)
        for kh in range(3):
            for t in range(T8):
                r = SROW[t] * CC
                for p in range(P6):
                    kw = t - p
                    if 0 <= kw <= 2:
                        s[r : r + CC, kh, p * CC : (p + 1) * CC] = (
                            w[:, :, kh, kw].T
                        )
        return s

    return s1.astype(BF16), mk_s(w2).astype(BF16), mk_s(w3).astype(BF16)


def _make_xT(xb, h0, cfg):
    """Per-core conv1 rhs: xT[(kh,t,ci), r1, j] = x[ci, h0-3+r1+kh, 6j+t-1],
    zero outside the image; rows r1 whose conv1-output image row is out of
    range are fully zeroed (so conv1 emits zero rows, matching the SAME-pad
    semantics of the following conv)."""
    CIN, H, W, SLAB = cfg["CIN"], cfg["H"], cfg["W"], cfg["SLAB"]
    JW = W // P6
    XROWS = SLAB + 4
    xp = np.zeros((CIN, XROWS + 2, W + T8), np.float32)
    rlo = max(0, h0 - 3)
    rhi = min(H, h0 + SLAB + 3)
    xp[:, rlo - (h0 - 3) : rhi - (h0 - 3), 1 : 1 + W] = xb[:, rlo:rhi, :]

    xT = np.empty((3, T8, CIN, XROWS, JW), np.float32)
    for kh in range(3):
        for t in range(T8):
            # w = 6j + t - 1  ->  padded col index 6j + t
            xT[kh, t] = xp[:, kh : kh + XROWS, t : t + 6 * JW : 6]
    xT = xT.reshape(3 * T8 * CIN, XROWS, JW)
    if h0 == 0:
        xT[:, 0:2, :] = 0.0
    if h0 + SLAB == H:
        xT[:, XROWS - 2 : XROWS, :] = 0.0
    return np.ascontiguousarray(xT).astype(BF16)


def _np_conv3x3(x, w):
    """Valid-in-H, SAME-in-W 3x3 conv.  x [ci, Rin, W] -> [co, Rin-2, W]."""
    ci, rin, W = x.shape
    xp = np.pad(x, ((0, 0), (0, 0), (1, 1)))
    out = np.zeros((w.shape[0], rin - 2, W), np.float32)
    for kh in range(3):
        for kw in range(3):
            out += np.einsum(
                "oi,irw->orw", w[:, :, kh, kw], xp[:, kh : kh + rin - 2, kw : kw + W]
            )
    return out


def _patch_edge_row(xb, w1, w2, w3, row, H):
    """Host-recompute output row `row` (0 or H-1) of one batch image."""
    strip = np.zeros((xb.shape[0], 7, xb.shape[2]), np.float32)
    if row == 0:
        strip[:, 3:7] = xb[:, 0:4]
        a1 = _np_conv3x3(strip, w1)        # rows -2..2
        a1[:, 0:2] = 0.0                   # SAME-pad zeros of conv2's input
        a2 = _np_conv3x3(a1, w2)           # rows -1..1
        a2[:, 0:1] = 0.0
        o = _np_conv3x3(a2, w3)            # row 0
    else:
        strip[:, 0:4] = xb[:, H - 4 : H]
        a1 = _np_conv3x3(strip, w1)        # rows H-3..H+1
        a1[:, 3:5] = 0.0
        a2 = _np_conv3x3(a1, w2)           # rows H-2..H
        a2[:, 2:3] = 0.0
        o = _np_conv3x3(a2, w3)            # row H-1
    return np.maximum(o[:, 0], 0.0)


# ------------------------------------------------------------- runners

def _install_ntff_hook():
    """Provide antenv.axon_hooks (absent in this image) so bass_utils can
    profile through the axon NTFF path."""
    import types

    if "antenv.axon_hooks" in sys.modules:
        return
    try:
        from trn_agent_boot.trn_boot import _ntff_profile_via_ctypes

        hook = _ntff_profile_via_ctypes("/opt/axon/libaxon_pjrt.so")
    except Exception:
        hook = None
    mod = types.ModuleType("antenv.axon_hooks")
    mod.get_axon_ntff_profile_hook = lambda: hook
    mod.set_axon_ntff_profile_hook = lambda h: None
    sys.modules["antenv.axon_hooks"] = mod


_PROGRAM_CACHE = {}


def _get_program():
    if "full" not in _PROGRAM_CACHE:
        _PROGRAM_CACHE["full"] = _build_program(FULL_CFG)
    return _PROGRAM_CACHE["full"]


def _make_in_maps(x, w1, w2, w3, cfg):
    SLAB, H = cfg["SLAB"], cfg["H"]
    halves = H // SLAB
    s1, s2, s3 = _make_stationaries(w1, w2, w3, cfg)
    in_maps = []
    for core in range(cfg["N_CORES"]):
        b, half = core // halves, core % halves
        in_maps.append(
            {"xT": _make_xT(x[b], half * SLAB, cfg), "s1": s1, "s2": s2, "s3": s3}
        )
    return in_maps


def _assemble(res, x, w1, w2, w3, cfg):
    B, H, W, SLAB, CC = cfg["B"], cfg["H"], cfg["W"], cfg["SLAB"], cfg["CC"]
    JW = W // P6
    halves = H // SLAB
    y = np.empty((B, CC, H, W), np.float32)
    for core in range(cfg["N_CORES"]):
        b, half = core // halves, core % halves
        h0 = half * SLAB
        r = np.asarray(res[core]["out"])  # [(p,co)=96, SLAB, JW]
        r = (
            r.reshape(P6, CC, SLAB, JW)
            .transpose(1, 2, 3, 0)
            .reshape(CC, SLAB, JW * P6)
        )
        y[b, :, h0 : h0 + SLAB, :] = r
    for b in range(B):
        y[b, :, 0, :] = _patch_edge_row(x[b], w1, w2, w3, 0, H)
        y[b, :, H - 1, :] = _patch_edge_row(x[b], w1, w2, w3, H - 1, H)
    return y


def _kernel_impl(x, w1, w2, w3, cfg, runner):
    in_maps = _make_in_maps(x, w1, w2, w3, cfg)
    res = runner(in_maps)
    return _assemble(res, x, w1, w2, w3, cfg)


def kernel(x, w1, w2, w3, H, W, Th, Tw):
    x = np.asarray(x, dtype=np.float32)
    w1 = np.asarray(w1, dtype=np.float32)
    w2 = np.asarray(w2, dtype=np.float32)
    w3 = np.asarray(w3, dtype=np.float32)
    cfg = dict(FULL_CFG)
    assert x.shape == (cfg["B"], cfg["CIN"], cfg["H"], cfg["W"]), x.shape

    nc = _get_program()

    def runner(in_maps):
        from concourse.bass_utils import run_bass_kernel_spmd

        return run_bass_kernel_spmd(
            nc, in_maps, list(range(cfg["N_CORES"]))
        ).results

    return _kernel_impl(x, w1, w2, w3, cfg, runner)


# revision 5
# speedup vs baseline: 1.5770x; 1.2556x over previous
"""Trainium2 Bass kernel for nn_Net_37048387896011.

Three stacked 3x3 SAME convs (3->16->16->16 channels) + final ReLU on a
[4, 3, 1536, 1536] fp32 image, distributed over 8 NeuronCores as
(batch x H-halves).  No collectives: each core's input slab carries a
3-row halo read from the full image on the host.

Math mapping (per conv layer): multi-pixel Toeplitz matmul.  The
stationary operand is [K=(t,ci)=128, M=(p,cout)=96] where t indexes an
8-wide input window along W and p indexes 6 output pixels; kernel rows
(kh) are accumulated into PSUM over 3 matmuls (conv1 instead folds kh
into K=72 using host-replicated input rows, one matmul per block).
Activations live in SBUF in a Toeplitz layout: the partition for (t,ci)
holds x[ci, :, 6j+t-1] over j-blocks of 6 pixels, so one matmul column
computes 6 output pixels x 16 channels with no on-device im2col.

SBUF layout per activation buffer [128, rows, JW+2]:
  partitions [0,96)    = t in 1..6 at (t-1)*16+ci   (written by PSUM evac
                         at partition offset 0 - engine APs must start at
                         partition 0/32/64/96)
  partitions [96,112)  = t=0   (dup of t=6 shifted one j-block)
  partitions [112,128) = t=7   (dup of t=1 shifted one j-block)
  free cols 0 and JW+1 are permanent zeros; j-data lives in [1, JW+1).
The dup groups are filled by two SBUF->SBUF DMAs per buffer per tile
(DMA is not subject to the partition-quadrant constraint), and the zero
columns make the W-edge SAME-padding zeros automatic.

The device writes conv3's output in its natural PSUM order
[(p,cout), h, j]; the host de-interleaves to [cout, h, w] and patches
the two image-edge rows (h=0, h=1535) whose device values saw a garbage
SAME-pad halo row.
"""

import sys

if "/opt/trn_rl_repo" not in sys.path:
    sys.path.insert(0, "/opt/trn_rl_repo")

import ml_dtypes
import numpy as np

BF16 = ml_dtypes.bfloat16

FULL_CFG = dict(
    B=4, CIN=3, CC=16, H=1536, W=1536,
    SLAB=768,     # output rows per core
    R=48,         # output rows per tile
    OC=8,         # output rows per out-DMA chunk
    N_CORES=8,
)

P6 = 6   # output pixels per matmul column
T8 = 8   # input window width (P6 + 2)

# SBUF partition row for Toeplitz slot t (times 16, plus ci):
#   t=1..6 -> 0..5 ;  t=0 -> 6 ;  t=7 -> 7
SROW = {1: 0, 2: 1, 3: 2, 4: 3, 5: 4, 6: 5, 0: 6, 7: 7}


def _build_program(cfg):
    """Build the (SPMD-identical) Bass program for one core."""
    import concourse.bacc as bacc
    import concourse.mybir as mybir
    from concourse.tile import TileContext

    W, SLAB, R, OC = cfg["W"], cfg["SLAB"], cfg["R"], cfg["OC"]
    CIN, CC = cfg["CIN"], cfg["CC"]
    JW = W // P6
    NT = SLAB // R
    K1 = 3 * T8 * CIN          # conv1 contraction: (kh, t, ci)
    M = 128                    # matmul output: (p, co) = 96, padded to 128
                               # so bf16 Fast Weight Load engages
    MOUT = P6 * CC             # the 96 meaningful output partitions
    XROWS = SLAB + 4           # conv1-output rows per core

    f32 = mybir.dt.float32
    bf16 = mybir.dt.bfloat16

    nc = bacc.Bacc(None, target_bir_lowering=False, debug=False)

    xT_d = nc.declare_dram_parameter("xT", [K1, XROWS, JW], bf16, isOutput=False)
    s1_d = nc.declare_dram_parameter("s1", [K1, M], bf16, isOutput=False)
    s2_d = nc.declare_dram_parameter("s2", [T8 * CC, 3, M], bf16, isOutput=False)
    s3_d = nc.declare_dram_parameter("s3", [T8 * CC, 3, M], bf16, isOutput=False)
    out_d = nc.declare_dram_parameter("out", [MOUT, SLAB, JW], f32, isOutput=True)

    with TileContext(nc) as tc:
        with (
            tc.tile_pool(name="weights", bufs=1) as wpool,
            tc.tile_pool(name="xt", bufs=2) as xtpool,
            tc.tile_pool(name="act", bufs=1) as actpool,
            tc.tile_pool(name="outsb", bufs=3) as outpool,
            tc.tile_pool(name="ps", bufs=8, space="PSUM") as pspool,
        ):
            s1_sb = wpool.tile([K1, M], bf16, tag="s1")
            nc.sync.dma_start(out=s1_sb[:], in_=s1_d[:])
            s2_sb = wpool.tile([T8 * CC, 3, M], bf16, tag="s2")
            nc.sync.dma_start(out=s2_sb[:], in_=s2_d[:])
            s3_sb = wpool.tile([T8 * CC, 3, M], bf16, tag="s3")
            nc.sync.dma_start(out=s3_sb[:], in_=s3_d[:])

            # double-buffered a1 (conv1 of tile g+1 overlaps conv2/3 of g)
            a1A = actpool.tile([T8 * CC, R + 4, JW + 2], bf16, tag="a1A")
            a1B = actpool.tile([T8 * CC, R + 4, JW + 2], bf16, tag="a1B")
            a2 = actpool.tile([T8 * CC, R + 2, JW + 2], bf16, tag="a2")
            for a in (a1A, a1B, a2):
                nc.gpsimd.memset(a[0 : 6 * CC, :, 0:1], 0.0)
                nc.gpsimd.memset(a[0 : 6 * CC, :, JW + 1 : JW + 2], 0.0)

            def fill_dups(a, rows):
                # Issued on the ACT HWDGE ring, separate from the bulk
                # (sync-ring) DMAs, so these latency-critical fills do not
                # queue behind megabyte transfers.
                #
                # Each dup group is one flat byte-shifted copy over the
                # whole [rows, JW+2] free range: the per-row column shift
                # falls out of the row-major layout, and the row-boundary
                # bleed lands in (or reads from) the permanent zero
                # columns.  One ~26KB contiguous descriptor per partition
                # instead of one 512B descriptor per row.
                L = rows * (JW + 2)
                # t=0 slots [96,112): t0(r, c) = t6(r, c-1)
                nc.scalar.dma_start(
                    out=a[6 * CC : 7 * CC, :, :].rearrange("p r c -> p (r c)")[:, 1:L],
                    in_=a[5 * CC : 6 * CC, :, :].rearrange("p r c -> p (r c)")[:, 0 : L - 1],
                )
                # t=7 slots [112,128): t7(r, c) = t1(r, c+1)
                nc.scalar.dma_start(
                    out=a[7 * CC : 8 * CC, :, :].rearrange("p r c -> p (r c)")[:, 0 : L - 1],
                    in_=a[0:CC, :, :].rearrange("p r c -> p (r c)")[:, 1:L],
                )

            def conv1_tile(g, a1):
                gb = g * R
                xt = xtpool.tile([K1, R + 4, JW], bf16, tag="xt",
                                 name=f"xt_{g}")
                nc.sync.dma_start(out=xt[:], in_=xT_d[:, gb : gb + R + 4, :])
                for i in range((R + 4) // 2):
                    r0 = 2 * i
                    ps = pspool.tile([M, 2, JW], f32, tag="ps",
                                     name=f"psa_{g}_{i}")
                    nc.tensor.matmul(
                        ps[:], lhsT=s1_sb[:], rhs=xt[:, r0 : r0 + 2, :],
                        start=True, stop=True,
                    )
                    # psum (p,co) -> partition p*16+co (t=p+1 slot)
                    nc.vector.tensor_copy(
                        a1[0 : 6 * CC, r0 : r0 + 2, 1 : JW + 1], ps[0:MOUT]
                    )
                fill_dups(a1, R + 4)

            def conv23_tile(g, a1):
                gb = g * R
                nblk2 = (R + 2) // 2
                for base in range(0, nblk2, 6):
                    blks = list(range(base, min(base + 6, nblk2)))
                    pss = [
                        pspool.tile([M, 2, JW], f32, tag="ps",
                                    name=f"psb_{g}_{base}_{_i}")
                        for _i in range(len(blks))
                    ]
                    for kh in range(3):
                        for ps, i in zip(pss, blks):
                            nc.tensor.matmul(
                                ps[:], lhsT=s2_sb[:, kh, :],
                                rhs=a1[:, 2 * i + kh : 2 * i + kh + 2, 1 : JW + 1],
                                start=(kh == 0), stop=(kh == 2),
                            )
                    for ps, i in zip(pss, blks):
                        nc.vector.tensor_copy(
                            a2[0 : 6 * CC, 2 * i : 2 * i + 2, 1 : JW + 1],
                            ps[0:MOUT],
                        )
                fill_dups(a2, R + 2)

                for s in range(R // OC):
                    osb = outpool.tile([MOUT, OC, JW], f32, tag="outsb",
                                       name=f"osb_{g}_{s}")
                    nblk = OC // 2
                    for base in range(0, nblk, 4):
                        blks = list(range(base, min(base + 4, nblk)))
                        pss = [
                            pspool.tile([M, 2, JW], f32, tag="ps",
                                        name=f"psc_{g}_{s}_{base}_{_i}")
                            for _i in range(len(blks))
                        ]
                        for kh in range(3):
                            for ps, i in zip(pss, blks):
                                ii = s * nblk + i
                                nc.tensor.matmul(
                                    ps[:], lhsT=s3_sb[:, kh, :],
                                    rhs=a2[:, 2 * ii + kh : 2 * ii + kh + 2,
                                           1 : JW + 1],
                                    start=(kh == 0), stop=(kh == 2),
                                )
                        for ps, i in zip(pss, blks):
                            nc.scalar.activation(
                                osb[:, 2 * i : 2 * i + 2, :], ps[0:MOUT],
                                mybir.ActivationFunctionType.Relu,
                            )
                    nc.sync.dma_start(
                        out=out_d[:, gb + s * OC : gb + (s + 1) * OC, :],
                        in_=osb[:],
                    )

            a1bufs = [a1A, a1B]
            conv1_tile(0, a1bufs[0])
            for g in range(NT):
                if g + 1 < NT:
                    conv1_tile(g + 1, a1bufs[(g + 1) % 2])
                conv23_tile(g, a1bufs[g % 2])

    nc.compile()
    return nc


# ------------------------------------------------------------ host prep

def _make_stationaries(w1, w2, w3, cfg):
    CIN, CC = cfg["CIN"], cfg["CC"]
    M = 128
    K1 = 3 * T8 * CIN

    # conv1: natural (kh, t, ci) K-order, matching xT
    s1 = np.zeros((K1, M), np.float32)
    for kh in range(3):
        for t in range(T8):
            for p in range(P6):
                kw = t - p
                if 0 <= kw <= 2:
                    s1[
                        kh * T8 * CIN + t * CIN : kh * T8 * CIN + (t + 1) * CIN,
                        p * CC : (p + 1) * CC,
                    ] = w1[:, :, kh, kw].T

    # conv2/3: SROW-permuted K-order matching the SBUF Toeplitz layout
    def mk_s(w):
        s = np.zeros((T8 * CC, 3, M), np.float32)
        for kh in range(3):
            for t in range(T8):
                r = SROW[t] * CC
                for p in range(P6):
                    kw = t - p
                    if 0 <= kw <= 2:
                        s[r : r + CC, kh, p * CC : (p + 1) * CC] = (
                            w[:, :, kh, kw].T
                        )
        return s

    return s1.astype(BF16), mk_s(w2).astype(BF16), mk_s(w3).astype(BF16)


def _make_xT(xb, h0, cfg):
    """Per-core conv1 rhs: xT[(kh,t,ci), r1, j] = x[ci, h0-3+r1+kh, 6j+t-1],
    zero outside the image; rows r1 whose conv1-output image row is out of
    range are fully zeroed (so conv1 emits zero rows, matching the SAME-pad
    semantics of the following conv)."""
    CIN, H, W, SLAB = cfg["CIN"], cfg["H"], cfg["W"], cfg["SLAB"]
    JW = W // P6
    XROWS = SLAB + 4
    xp = np.zeros((CIN, XROWS + 2, W + T8), np.float32)
    rlo = max(0, h0 - 3)
    rhi = min(H, h0 + SLAB + 3)
    xp[:, rlo - (h0 - 3) : rhi - (h0 - 3), 1 : 1 + W] = xb[:, rlo:rhi, :]

    xT = np.empty((3, T8, CIN, XROWS, JW), np.float32)
    for kh in range(3):
        for t in range(T8):
            # w = 6j + t - 1  ->  padded col index 6j + t
            xT[kh, t] = xp[:, kh : kh + XROWS, t : t + 6 * JW : 6]
    xT = xT.reshape(3 * T8 * CIN, XROWS, JW)
    if h0 == 0:
        xT[:, 0:2, :] = 0.0
    if h0 + SLAB == H:
        xT[:, XROWS - 2 : XROWS, :] = 0.0
    return np.ascontiguousarray(xT).astype(BF16)


def _np_conv3x3(x, w):
    """Valid-in-H, SAME-in-W 3x3 conv.  x [ci, Rin, W] -> [co, Rin-2, W]."""
    ci, rin, W = x.shape
    xp = np.pad(x, ((0, 0), (0, 0), (1, 1)))
    out = np.zeros((w.shape[0], rin - 2, W), np.float32)
    for kh in range(3):
        for kw in range(3):
            out += np.einsum(
                "oi,irw->orw", w[:, :, kh, kw], xp[:, kh : kh + rin - 2, kw : kw + W]
            )
    return out


def _patch_edge_row(xb, w1, w2, w3, row, H):
    """Host-recompute output row `row` (0 or H-1) of one batch image."""
    strip = np.zeros((xb.shape[0], 7, xb.shape[2]), np.float32)
    if row == 0:
        strip[:, 3:7] = xb[:, 0:4]
        a1 = _np_conv3x3(strip, w1)        # rows -2..2
        a1[:, 0:2] = 0.0                   # SAME-pad zeros of conv2's input
        a2 = _np_conv3x3(a1, w2)           # rows -1..1
        a2[:, 0:1] = 0.0
        o = _np_conv3x3(a2, w3)            # row 0
    else:
        strip[:, 0:4] = xb[:, H - 4 : H]
        a1 = _np_conv3x3(strip, w1)        # rows H-3..H+1
        a1[:, 3:5] = 0.0
        a2 = _np_conv3x3(a1, w2)           # rows H-2..H
        a2[:, 2:3] = 0.0
        o = _np_conv3x3(a2, w3)            # row H-1
    return np.maximum(o[:, 0], 0.0)


# ------------------------------------------------------------- runners

def _install_ntff_hook():
    """Provide antenv.axon_hooks (absent in this image) so bass_utils can
    profile through the axon NTFF path."""
    import types

    if "antenv.axon_hooks" in sys.modules:
        return
    try:
        from trn_agent_boot.trn_boot import _ntff_profile_via_ctypes

        hook = _ntff_profile_via_ctypes("/opt/axon/libaxon_pjrt.so")
    except Exception:
        hook = None
    mod = types.ModuleType("antenv.axon_hooks")
    mod.get_axon_ntff_profile_hook = lambda: hook
    mod.set_axon_ntff_profile_hook = lambda h: None
    sys.modules["antenv.axon_hooks"] = mod


_PROGRAM_CACHE = {}


def _get_program():
    if "full" not in _PROGRAM_CACHE:
        _PROGRAM_CACHE["full"] = _build_program(FULL_CFG)
    return _PROGRAM_CACHE["full"]


def _make_in_maps(x, w1, w2, w3, cfg):
    SLAB, H = cfg["SLAB"], cfg["H"]
    halves = H // SLAB
    s1, s2, s3 = _make_stationaries(w1, w2, w3, cfg)
    in_maps = []
    for core in range(cfg["N_CORES"]):
        b, half = core // halves, core % halves
        in_maps.append(
            {"xT": _make_xT(x[b], half * SLAB, cfg), "s1": s1, "s2": s2, "s3": s3}
        )
    return in_maps


def _assemble(res, x, w1, w2, w3, cfg):
    B, H, W, SLAB, CC = cfg["B"], cfg["H"], cfg["W"], cfg["SLAB"], cfg["CC"]
    JW = W // P6
    halves = H // SLAB
    y = np.empty((B, CC, H, W), np.float32)
    for core in range(cfg["N_CORES"]):
        b, half = core // halves, core % halves
        h0 = half * SLAB
        r = np.asarray(res[core]["out"])  # [(p,co)=96, SLAB, JW]
        r = (
            r.reshape(P6, CC, SLAB, JW)
            .transpose(1, 2, 3, 0)
            .reshape(CC, SLAB, JW * P6)
        )
        y[b, :, h0 : h0 + SLAB, :] = r
    for b in range(B):
        y[b, :, 0, :] = _patch_edge_row(x[b], w1, w2, w3, 0, H)
        y[b, :, H - 1, :] = _patch_edge_row(x[b], w1, w2, w3, H - 1, H)
    return y


def _kernel_impl(x, w1, w2, w3, cfg, runner):
    in_maps = _make_in_maps(x, w1, w2, w3, cfg)
    res = runner(in_maps)
    return _assemble(res, x, w1, w2, w3, cfg)


def kernel(x, w1, w2, w3, H, W, Th, Tw):
    x = np.asarray(x, dtype=np.float32)
    w1 = np.asarray(w1, dtype=np.float32)
    w2 = np.asarray(w2, dtype=np.float32)
    w3 = np.asarray(w3, dtype=np.float32)
    cfg = dict(FULL_CFG)
    assert x.shape == (cfg["B"], cfg["CIN"], cfg["H"], cfg["W"]), x.shape

    nc = _get_program()

    def runner(in_maps):
        from concourse.bass_utils import run_bass_kernel_spmd

        return run_bass_kernel_spmd(
            nc, in_maps, list(range(cfg["N_CORES"]))
        ).results

    return _kernel_impl(x, w1, w2, w3, cfg, runner)
